# revision 14
# baseline (speedup 1.0000x reference)
"""Trainium2 Bass kernel for nn_Part_Block (SE-style dynamic-weight CNN block).

Computation (per batch b):
    pooled = mean_hw x[b]                       (C,)
    hidden = silu(pooled @ fc1_w.T + fc1_b)     (128,)
    dw     = (hidden @ fc2_w.T + fc2_b)         (P*C,) -> (P, C)
    base   = x[b] * conv_w + conv_b             (C, H, W)
    out    = softmax_p( einsum('chw,pc->phw', base, dw) )

Device strategy (8 cores, data-parallel over batch, 4 batches/core):
    - conv folded into fc2:  logits[p,hw] = sum_c x[c,hw]*(conv_w[c]*dw[p,c]) + beta[p]
      with beta[p] = g[p,:] @ hidden + d[p]      (g, d host-precomputed)
    - fc2 emitted as 64 [128h x 128c] column-tiles so PE directly produces
      dwsT[c, (t,p,b)] (the einsum lhsT) with no transpose.
    - x shipped as bf16 (halves upload), einsum in bf16, fc32 accumulate.

This environment executes bass NEFFs at ~30-50us *per instruction*
(bytes are comparatively free), so the kernel is structured to minimize
instruction count: one pooling reduce for all 4 batches, fc1/fc2/beta
batched over the 4 batches (16+64+1 matmuls total), einsum 32 matmuls
per batch (the PE streaming floor given N<=512/PSUM bank), softmax via
Exp(+beta bias) on ScalarE + ones-matmul column sums + reciprocal +
ones-matmul partition-broadcast + muls, single output DMA.
"""

from contextlib import ExitStack

import ml_dtypes
import numpy as np

import concourse.bass as bass
import concourse.mybir as mybir
import concourse.tile as tile
from concourse import bacc
from concourse.bass_utils import run_bass_kernel_spmd

N_CORES = 8
B, C, H, W = 32, 2048, 24, 24
HW = H * W                      # 576
P = 4                           # parts
B_LOC = B // N_CORES            # 4 batches per core
NT = C // 128                   # 16 c-tiles
NQ = P * NT                     # 64 fc2 column-tiles
NS = 288                        # einsum N split (576 = 2*288)

F32 = mybir.dt.float32
BF16 = mybir.dt.bfloat16

_BUILD_CACHE: dict = {}


def _build(repeat: int = 1):
    """Build + compile the SPMD single-core program (same on all 8 cores)."""
    nc = bacc.Bacc(
        "TRN2", target_bir_lowering=False, debug=False, num_devices=N_CORES
    )
    xs = nc.dram_tensor("xs", [B_LOC, C, HW], BF16, kind="ExternalInput")
    fc1w_d = nc.dram_tensor("fc1w", [128, C], BF16, kind="ExternalInput")
    fc1b_d = nc.dram_tensor("fc1b", [128, 1], F32, kind="ExternalInput")
    fc2w_d = nc.dram_tensor("fc2w", [128, NQ * 128], BF16, kind="ExternalInput")
    fc2bs_d = nc.dram_tensor("fc2bs", [128, NQ * P], F32, kind="ExternalInput")
    gt_d = nc.dram_tensor("gt", [128, P], BF16, kind="ExternalInput")
    d_d = nc.dram_tensor("dvec", [P, 1], F32, kind="ExternalInput")
    ys = nc.dram_tensor("ys", [B_LOC, P, HW], F32, kind="ExternalOutput")

    with tile.TileContext(nc) as tc:
        with ExitStack() as ctx:
            const = ctx.enter_context(tc.tile_pool(name="const", bufs=1))
            data = ctx.enter_context(tc.tile_pool(name="data", bufs=1))
            psum = ctx.enter_context(tc.tile_pool(name="ps", bufs=1, space="PSUM"))

            fc1w = const.tile([128, C], BF16)
            nc.sync.dma_start(fc1w[:], fc1w_d.ap())
            fc1b = const.tile([128, 1], F32)
            nc.sync.dma_start(fc1b[:], fc1b_d.ap())
            fc2w = const.tile([128, NQ * 128], BF16)
            nc.sync.dma_start(fc2w[:], fc2w_d.ap())
            fc2bs = const.tile([128, NQ * P], F32)
            nc.sync.dma_start(fc2bs[:], fc2bs_d.ap())
            gt = const.tile([128, P], BF16)
            nc.sync.dma_start(gt[:], gt_d.ap())
            dvec = const.tile([P, 1], F32)
            nc.sync.dma_start(dvec[:], d_d.ap())
            ones4 = const.tile([P, 1], F32)
            nc.vector.memset(ones4[:], 1.0)
            ones1x4 = const.tile([1, P], F32)
            nc.vector.memset(ones1x4[:], 1.0)

            for _ in range(repeat):
                # ---- load x (bf16): x_sb[p, (b*16+t)*576+f] = x[b, t*128+p, f]
                x_sb = data.tile([128, B_LOC * NT * HW], BF16)
                for b in range(B_LOC):
                    nc.sync.dma_start(
                        x_sb[:, b * NT * HW : (b + 1) * NT * HW],
                        xs.ap()[b].rearrange("(t p) f -> p t f", p=128),
                    )
                # ---- pooling for all batches in one reduce: col g = b*16+t
                pooled = data.tile([128, B_LOC * NT], F32)
                nc.vector.reduce_sum(
                    pooled[:],
                    x_sb[:].rearrange("p (g f) -> p g f", f=HW),
                    mybir.AxisListType.X,
                )
                pooled_bf = data.tile([128, B_LOC * NT], BF16)
                nc.vector.tensor_copy(pooled_bf[:], pooled[:])
                pbv = pooled_bf[:].rearrange("p (b t) -> p b t", t=NT)

                # ---- fc1 batched over b: psum cols 0:4; beta in cols 4:8
                mm_ps = psum.tile([128, 2 * P], F32)
                for t in range(NT):
                    nc.tensor.matmul(
                        mm_ps[:, 0:P],
                        lhsT=fc1w[:, t * 128 : (t + 1) * 128],
                        rhs=pbv[:, :, t],
                        start=(t == 0),
                        stop=(t == NT - 1),
                    )
                hidden = data.tile([128, P], BF16)
                nc.scalar.activation(
                    hidden[:],
                    mm_ps[:, 0:P],
                    mybir.ActivationFunctionType.Silu,
                    bias=fc1b[:, 0:1],
                    scale=1.0 / HW,
                )
                # ---- beta[p, b] = gT.T @ hidden + d
                nc.tensor.matmul(
                    mm_ps[0:P, P : 2 * P],
                    lhsT=gt[:],
                    rhs=hidden[:],
                    start=True,
                    stop=True,
                )
                beta = data.tile([P, P], F32)
                nc.scalar.activation(
                    beta[:],
                    mm_ps[0:P, P : 2 * P],
                    mybir.ActivationFunctionType.Identity,
                    bias=dvec[:, 0:1],
                )

                # ---- fc2 batched: dwsT[c, (t,p,b)] col (t*4+p)*4+b
                fc2_ps = psum.tile([128, NQ * P], F32)
                for q in range(NQ):
                    nc.tensor.matmul(
                        fc2_ps[:, q * P : (q + 1) * P],
                        lhsT=fc2w[:, q * 128 : (q + 1) * 128],
                        rhs=hidden[:],
                        start=True,
                        stop=True,
                    )
                dwst = data.tile([128, NQ * P], BF16)
                nc.vector.tensor_add(dwst[:], fc2_ps[:], fc2bs[:])
                dwv = dwst[:].rearrange("c (t p b) -> c t p b", p=P, b=B_LOC)

                # ---- per batch: einsum + softmax
                e_sb = data.tile([P, B_LOC * HW], F32)
                r_sb = data.tile([1, B_LOC * HW], F32)
                out_sb = data.tile([P, B_LOC * HW], F32)
                for b in range(B_LOC):
                    ps_a = psum.tile([P, NS], F32)
                    ps_b = psum.tile([P, NS], F32)
                    xo = (b * NT) * HW
                    for t in range(NT):
                        lw = dwv[:, t, :, b]
                        nc.tensor.matmul(
                            ps_a[:],
                            lhsT=lw,
                            rhs=x_sb[:, xo + t * HW : xo + t * HW + NS],
                            start=(t == 0),
                            stop=(t == NT - 1),
                        )
                        nc.tensor.matmul(
                            ps_b[:],
                            lhsT=lw,
                            rhs=x_sb[:, xo + t * HW + NS : xo + (t + 1) * HW],
                            start=(t == 0),
                            stop=(t == NT - 1),
                        )
                    eo = b * HW
                    nc.scalar.activation(
                        e_sb[:, eo : eo + NS],
                        ps_a[:],
                        mybir.ActivationFunctionType.Exp,
                        bias=beta[:, b : b + 1],
                    )
                    nc.scalar.activation(
                        e_sb[:, eo + NS : eo + HW],
                        ps_b[:],
                        mybir.ActivationFunctionType.Exp,
                        bias=beta[:, b : b + 1],
                    )
                    cs_a = psum.tile([1, NS], F32)
                    cs_b = psum.tile([1, NS], F32)
                    nc.tensor.matmul(
                        cs_a[:], lhsT=ones4[:], rhs=e_sb[:, eo : eo + NS],
                        start=True, stop=True,
                    )
                    nc.tensor.matmul(
                        cs_b[:], lhsT=ones4[:], rhs=e_sb[:, eo + NS : eo + HW],
                        start=True, stop=True,
                    )
                    nc.vector.reciprocal(r_sb[:, eo : eo + NS], cs_a[:])
                    nc.vector.reciprocal(r_sb[:, eo + NS : eo + HW], cs_b[:])
                    r4_a = psum.tile([P, NS], F32)
                    r4_b = psum.tile([P, NS], F32)
                    nc.tensor.matmul(
                        r4_a[:], lhsT=ones1x4[:], rhs=r_sb[0:1, eo : eo + NS],
                        start=True, stop=True,
                    )
                    nc.tensor.matmul(
                        r4_b[:], lhsT=ones1x4[:], rhs=r_sb[0:1, eo + NS : eo + HW],
                        start=True, stop=True,
                    )
                    nc.vector.tensor_mul(
                        out_sb[:, eo : eo + NS], e_sb[:, eo : eo + NS], r4_a[:]
                    )
                    nc.vector.tensor_mul(
                        out_sb[:, eo + NS : eo + HW], e_sb[:, eo + NS : eo + HW],
                        r4_b[:],
                    )
                nc.sync.dma_start(
                    ys.ap().rearrange("b p f -> p b f"), out_sb[:]
                )
    nc.compile()
    return nc


def _host_prep(fc1_w, fc1_b, fc2_w, fc2_b, conv_w, conv_b):
    """Precompute device weight layouts on host (all small tensors)."""
    fc1_w = np.asarray(fc1_w, np.float32)
    fc1_b = np.asarray(fc1_b, np.float32)
    fc2_w = np.asarray(fc2_w, np.float32)
    fc2_b = np.asarray(fc2_b, np.float32)
    conv_w = np.asarray(conv_w, np.float32)
    conv_b = np.asarray(conv_b, np.float32)

    # fc1w[p, t*128+j] = fc1_w[j, t*128+p]
    fc1w = np.ascontiguousarray(
        fc1_w.reshape(128, NT, 128).transpose(2, 1, 0).reshape(128, C)
    ).astype(ml_dtypes.bfloat16)
    # fc2 scaled by conv_w over channel:  fc2_ws[i, h] = fc2_w[i, h]*conv_w[i % C]
    fc2_ws = fc2_w * np.tile(conv_w, P)[:, None]
    # fc2w[h, (t*4+p)*128 + c] = fc2_ws[p*C + t*128 + c, h]
    fc2wt = np.ascontiguousarray(
        fc2_ws.reshape(P, NT, 128, 128).transpose(3, 1, 0, 2).reshape(128, NQ * 128)
    ).astype(ml_dtypes.bfloat16)
    # fc2bs[c, (t*4+p)*4+b] = fc2_b[p*C + t*128 + c]*conv_w[t*128+c]  (b-independent)
    fc2_bs = fc2_b * np.tile(conv_w, P)
    fc2bs = np.repeat(
        fc2_bs.reshape(P, NT, 128).transpose(2, 1, 0).reshape(128, NQ), P, axis=1
    ).astype(np.float32)
    # g[p, h] = sum_c conv_b[c]*fc2_w[p*C+c, h];  d[p] = sum_c conv_b[c]*fc2_b[p*C+c]
    g = (fc2_w.reshape(P, C, 128).astype(np.float64) *
         conv_b.astype(np.float64)[None, :, None]).sum(axis=1)
    gt = np.ascontiguousarray(g.T).astype(ml_dtypes.bfloat16)
    d = (fc2_b.reshape(P, C).astype(np.float64) @ conv_b.astype(np.float64))
    dvec = d.astype(np.float32).reshape(P, 1)
    return {
        "fc1w": fc1w,
        "fc1b": fc1_b.reshape(128, 1),
        "fc2w": fc2wt,
        "fc2bs": np.ascontiguousarray(fc2bs),
        "gt": gt,
        "dvec": dvec,
    }


def _run(in_maps, repeat: int = 1):
    if repeat not in _BUILD_CACHE:
        _BUILD_CACHE[repeat] = _build(repeat)
    nc = _BUILD_CACHE[repeat]
    return run_bass_kernel_spmd(nc, in_maps, list(range(N_CORES)))


def make_in_maps(x, weights):
    x3 = np.asarray(x, np.float32).reshape(B, C, HW).astype(ml_dtypes.bfloat16)
    return [
        {"xs": x3[i * B_LOC : (i + 1) * B_LOC], **weights} for i in range(N_CORES)
    ]


def kernel(x, fc1_w, fc1_b, fc2_w, fc2_b, conv_w, conv_b):
    weights = _host_prep(fc1_w, fc1_b, fc2_w, fc2_b, conv_w, conv_b)
    in_maps = make_in_maps(x, weights)
    res = _run(in_maps, repeat=1)
    out = np.concatenate(
        [res.results[i]["ys"] for i in range(N_CORES)], axis=0
    )
    return np.ascontiguousarray(out.reshape(B, P, H, W).astype(np.float32))


# revision 15
# speedup vs baseline: 1.3896x; 1.3896x over previous
"""Trainium2 Bass kernel for nn_Part_Block (SE-style dynamic-weight CNN block).

Computation (per batch b):
    pooled = mean_hw x[b]                       (C,)
    hidden = silu(pooled @ fc1_w.T + fc1_b)     (128,)
    dw     = (hidden @ fc2_w.T + fc2_b)         (P*C,) -> (P, C)
    base   = x[b] * conv_w + conv_b             (C, H, W)
    out    = softmax_p( einsum('chw,pc->phw', base, dw) )

Sharding: data-parallel over batch across the 8 cores (4 batches/core),
no collectives.  The depthwise conv is folded into the dynamic weights:
    logits[p,hw] = sum_c x[c,hw] * (conv_w[c]*dw[p,c]) + beta[p]
    beta[p]      = sum_c conv_b[c]*dw[p,c]
so `base` is never materialized and x is read once.

Placement: this backend executes bass NEFFs with a large flat cost per
instruction (~35-70us), a ~10ns/element cost on vector/scalar-engine
ops, and a fast (BLAS-like) path only for matmuls with contiguous
operands.  The SE "squeeze" path (global pool + two tiny FCs, 0.13% of
the FLOPs) is therefore computed on the host in fp32, and the device
kernel does the heavy data-parallel part: the (2048 -> 4)
channel-weighted reduction over the full 151MB activation tensor as PE
matmuls (per batch: 16 K-tiles x 2 N-halves, contiguous operands), plus
the softmax over parts (Exp with per-part bias on ScalarE, ones-matmul
column sums, reciprocal, ones-matmul partition broadcast, multiply).
x is shipped as bf16 (halves the upload; logit error ~1e-4 relative).
"""

from contextlib import ExitStack

import ml_dtypes
import numpy as np

import concourse.bass as bass
import concourse.mybir as mybir
import concourse.tile as tile
from concourse import bacc
from concourse.bass_utils import run_bass_kernel_spmd

N_CORES = 8
B, C, H, W = 32, 2048, 24, 24
HW = H * W                      # 576
P = 4                           # parts
B_LOC = B // N_CORES            # 4 batches per core
NT = C // 128                   # 16 c-tiles
NS = 288                        # einsum N split (576 = 2*288)

F32 = mybir.dt.float32
BF16 = mybir.dt.bfloat16

_BUILD_CACHE: dict = {}


def _build(repeat: int = 1):
    """Build + compile the SPMD single-core program (same on all 8 cores)."""
    nc = bacc.Bacc(
        "TRN2", target_bir_lowering=False, debug=False, num_devices=N_CORES
    )
    xs = nc.dram_tensor("xs", [B_LOC, C, HW], BF16, kind="ExternalInput")
    # dwst[c', b*64 + t*4 + p] = conv_w[t*128+c'] * dw[b, p, t*128+c']
    dwst_d = nc.dram_tensor("dwst", [128, B_LOC * NT * P], BF16, kind="ExternalInput")
    # beta[p, b]
    beta_d = nc.dram_tensor("beta", [P, B_LOC], F32, kind="ExternalInput")
    ys = nc.dram_tensor("ys", [B_LOC, P, HW], F32, kind="ExternalOutput")

    with tile.TileContext(nc) as tc:
        with ExitStack() as ctx:
            const = ctx.enter_context(tc.tile_pool(name="const", bufs=1))
            data = ctx.enter_context(tc.tile_pool(name="data", bufs=1))
            psum = ctx.enter_context(tc.tile_pool(name="ps", bufs=1, space="PSUM"))

            ones4 = const.tile([P, 1], F32)
            nc.vector.memset(ones4[:], 1.0)
            ones1x4 = const.tile([1, P], F32)
            nc.vector.memset(ones1x4[:], 1.0)

            for _ in range(repeat):
                dwst = data.tile([128, B_LOC * NT * P], BF16)
                nc.sync.dma_start(dwst[:], dwst_d.ap())
                beta = data.tile([P, B_LOC], F32)
                nc.sync.dma_start(beta[:], beta_d.ap())
                # x_sb[p, (b*16+t)*576+f] = x[b, t*128+p, f]
                x_sb = data.tile([128, B_LOC * NT * HW], BF16)
                for b in range(B_LOC):
                    nc.sync.dma_start(
                        x_sb[:, b * NT * HW : (b + 1) * NT * HW],
                        xs.ap()[b].rearrange("(t p) f -> p t f", p=128),
                    )

                e_sb = data.tile([P, B_LOC * HW], F32)
                r_sb = data.tile([1, B_LOC * HW], F32)
                out_sb = data.tile([P, B_LOC * HW], F32)
                for b in range(B_LOC):
                    # ---- einsum: logits[p, hw] = sum_t dwsT_t.T @ x_t
                    ps_a = psum.tile([P, NS], F32)
                    ps_b = psum.tile([P, NS], F32)
                    xo = (b * NT) * HW
                    for t in range(NT):
                        lw = dwst[:, b * NT * P + t * P : b * NT * P + (t + 1) * P]
                        nc.tensor.matmul(
                            ps_a[:],
                            lhsT=lw,
                            rhs=x_sb[:, xo + t * HW : xo + t * HW + NS],
                            start=(t == 0),
                            stop=(t == NT - 1),
                        )
                        nc.tensor.matmul(
                            ps_b[:],
                            lhsT=lw,
                            rhs=x_sb[:, xo + t * HW + NS : xo + (t + 1) * HW],
                            start=(t == 0),
                            stop=(t == NT - 1),
                        )
                    # ---- softmax over p: e = exp(logits + beta)
                    eo = b * HW
                    nc.scalar.activation(
                        e_sb[:, eo : eo + NS],
                        ps_a[:],
                        mybir.ActivationFunctionType.Exp,
                        bias=beta[:, b : b + 1],
                    )
                    nc.scalar.activation(
                        e_sb[:, eo + NS : eo + HW],
                        ps_b[:],
                        mybir.ActivationFunctionType.Exp,
                        bias=beta[:, b : b + 1],
                    )
                    cs_a = psum.tile([1, NS], F32)
                    cs_b = psum.tile([1, NS], F32)
                    nc.tensor.matmul(
                        cs_a[:], lhsT=ones4[:], rhs=e_sb[:, eo : eo + NS],
                        start=True, stop=True,
                    )
                    nc.tensor.matmul(
                        cs_b[:], lhsT=ones4[:], rhs=e_sb[:, eo + NS : eo + HW],
                        start=True, stop=True,
                    )
                    nc.vector.reciprocal(r_sb[:, eo : eo + NS], cs_a[:])
                    nc.vector.reciprocal(r_sb[:, eo + NS : eo + HW], cs_b[:])
                    r4_a = psum.tile([P, NS], F32)
                    r4_b = psum.tile([P, NS], F32)
                    nc.tensor.matmul(
                        r4_a[:], lhsT=ones1x4[:], rhs=r_sb[0:1, eo : eo + NS],
                        start=True, stop=True,
                    )
                    nc.tensor.matmul(
                        r4_b[:], lhsT=ones1x4[:], rhs=r_sb[0:1, eo + NS : eo + HW],
                        start=True, stop=True,
                    )
                    nc.vector.tensor_mul(
                        out_sb[:, eo : eo + NS], e_sb[:, eo : eo + NS], r4_a[:]
                    )
                    nc.vector.tensor_mul(
                        out_sb[:, eo + NS : eo + HW], e_sb[:, eo + NS : eo + HW],
                        r4_b[:],
                    )
                nc.sync.dma_start(
                    ys.ap().rearrange("b p f -> p b f"), out_sb[:]
                )
    nc.compile()
    return nc


def _host_se(x3, fc1_w, fc1_b, fc2_w, fc2_b, conv_w, conv_b):
    """SE squeeze path on host (fp32/fp64, tiny): returns dwst (B,128,64) bf16
    and betaT (B, P) f32."""
    pooled = x3.mean(axis=2, dtype=np.float64)                    # (B, C)
    z = pooled @ fc1_w.astype(np.float64).T + fc1_b.astype(np.float64)
    hidden = z / (1.0 + np.exp(-z))                               # silu, (B, 128)
    dw = hidden @ fc2_w.astype(np.float64).T + fc2_b.astype(np.float64)  # (B, P*C)
    dws = dw * np.tile(conv_w.astype(np.float64), P)              # conv_w folded
    # dwst[b][c', t*4+p] = dws[b, p*C + t*128 + c']
    dwst = np.ascontiguousarray(
        dws.reshape(B, P, NT, 128).transpose(0, 3, 2, 1).reshape(B, 128, NT * P)
    ).astype(ml_dtypes.bfloat16)
    beta = dw.reshape(B, P, C) @ conv_b.astype(np.float64)        # (B, P)
    return dwst, beta.astype(np.float32)


def _run(in_maps, repeat: int = 1):
    if repeat not in _BUILD_CACHE:
        _BUILD_CACHE[repeat] = _build(repeat)
    nc = _BUILD_CACHE[repeat]
    return run_bass_kernel_spmd(nc, in_maps, list(range(N_CORES)))


def make_in_maps(x, fc1_w, fc1_b, fc2_w, fc2_b, conv_w, conv_b):
    x = np.asarray(x, np.float32)
    x3 = x.reshape(B, C, HW)
    xbf = x3.astype(ml_dtypes.bfloat16)
    dwst, beta = _host_se(
        x3,
        np.asarray(fc1_w, np.float32), np.asarray(fc1_b, np.float32),
        np.asarray(fc2_w, np.float32), np.asarray(fc2_b, np.float32),
        np.asarray(conv_w, np.float32), np.asarray(conv_b, np.float32),
    )
    in_maps = []
    for i in range(N_CORES):
        sl = slice(i * B_LOC, (i + 1) * B_LOC)
        in_maps.append({
            "xs": xbf[sl],
            "dwst": np.ascontiguousarray(
                dwst[sl].transpose(1, 0, 2).reshape(128, B_LOC * NT * P)
            ),
            "beta": np.ascontiguousarray(beta[sl].T),
        })
    return in_maps


def kernel(x, fc1_w, fc1_b, fc2_w, fc2_b, conv_w, conv_b):
    in_maps = make_in_maps(x, fc1_w, fc1_b, fc2_w, fc2_b, conv_w, conv_b)
    res = _run(in_maps, repeat=1)
    out = np.concatenate(
        [res.results[i]["ys"] for i in range(N_CORES)], axis=0
    )
    return np.ascontiguousarray(out.reshape(B, P, H, W).astype(np.float32))
